# revision 57
# baseline (speedup 1.0000x reference)
"""Multi-head attention (B=2, SQ=SK=2048, D=1024, H=16, DK=64) on 8 TRN2 cores.

Sharding: core c handles batch b = c//4 and head-group hg = c%4 (4 heads,
256 feature columns of each projection).  Each core computes its heads'
Q/K/V projections, causal+padding-masked softmax attention, and a partial
output projection; the host sums the 4 partials per batch.

v4: token-chunk software pipeline.  The per-512-token-chunk order is
  V-proj(tc) -> K-proj(tc) -> Q-proj(tc) -> O-proj(tc-1) -> attention(tc)
which is exact because causal attention for q-chunk tc only consumes
K/V tokens <= (tc+1)*512.  This overlaps the exp stream (scalar engine,
the co-bottleneck) with projection matmuls instead of serializing all
projections first.

All matmul operands are bf16 (fp32 PSUM accumulation); the host
pre-converts inputs and upconverts/sums the bf16 output partials.
Exact causal trimming: the 4 diagonal ktiles of every q-chunk only
compute the valid q tail (N = 512-128*d); the partial 128x128 diagonal
block is masked by a DVE multiply with a constant lower-triangle tile.

Device layouts (per core):
  qT/kT  [dk, tok]    dk on partitions, produced directly by the projection
  v      [tok, dk]    natural, padding mask folded into the rows plus a
                      "masked ones" column per head (the ones column makes
                      the ctxT matmul emit the softmax denominator for free)
  sT     [ktok, qtok] transposed scores (PSUM)
  pT     exp(sT/8)    SBUF bf16
  ctxT   [dk+1, qtok] accumulated over ktok tiles (last row = denominator)
  out    [qtok, D]    ctxT is the stationary operand, both sides natural

Softmax runs without max subtraction (scores are O(6) for randn inputs, so
exp cannot overflow).  Padding is exact: masked keys contribute exactly
zero to numerator and denominator, and all-masked rows produce ~0 output
(matching the reference's nan_to_num) via a tiny epsilon in the ones
column.
"""

import numpy as np

B, SQ, SK, D, H, DK = 2, 2048, 2048, 1024, 16, 64
N_CORES = 8
CORES_PER_BATCH = 4
DKC = D // CORES_PER_BATCH          # 256 projection columns per core
QCH = 512                           # q-chunk (moving free dim)
ONES_EPS = 1e-20

_PROG_CACHE = {}


def _build(cfg):
    """Build the per-core Bass program. cfg = (sq, sk, d, dkc)."""
    import concourse.bass as bass  # noqa: F401
    import concourse.mybir as mybir
    import concourse.tile as tile
    from concourse import bacc
    from contextlib import ExitStack

    f32 = mybir.dt.float32
    bf16 = mybir.dt.bfloat16
    i32 = mybir.dt.int32
    Exp = mybir.ActivationFunctionType.Exp
    mult = mybir.AluOpType.mult
    is_ge = mybir.AluOpType.is_ge

    sq, sk, d, dkc = cfg
    kc_n = d // 128                  # contraction chunks for projections
    mc_n = dkc // 128                # 128-wide dk chunks (q/k layout)
    kt_n = sk // 128                 # key tiles
    qc_n = sq // QCH                 # q chunks
    hpc = dkc // DK                  # heads per core
    vw = DK + 1                      # v row width per head incl. ones col
    fc_n = d // 512                  # output feature chunks

    nc = bacc.Bacc("TRN2", target_bir_lowering=False, debug=False,
                   enable_asserts=False, num_devices=N_CORES)

    # all inputs are pre-arranged on the host into the exact device layout
    # so every DMA is a single fully-contiguous descriptor (>=4KB runs per
    # partition; the former rearrange patterns ran the DMA engines at half
    # rate on 512B-1KB runs)
    xqT = nc.dram_tensor("xqT", [128, sq // 512, 4, kc_n, 128], bf16,
                         kind="ExternalInput").ap()
    xkT = nc.dram_tensor("xkT", [128, sk // 512, 4, kc_n, 128], bf16,
                         kind="ExternalInput").ap()
    xvT = nc.dram_tensor("xvT", [128, sk // 512, 4, kc_n, 128], bf16,
                         kind="ExternalInput").ap()
    wq_d = nc.dram_tensor("wq", [128, kc_n, dkc], bf16,
                          kind="ExternalInput").ap()
    wk_d = nc.dram_tensor("wk", [128, kc_n, dkc], bf16,
                          kind="ExternalInput").ap()
    wv_d = nc.dram_tensor("wv", [128, kc_n, dkc], bf16,
                          kind="ExternalInput").ap()
    wo_d = nc.dram_tensor("wo", [128, mc_n, fc_n, 512], bf16,
                          kind="ExternalInput").ap()
    mask_d = nc.dram_tensor("maskb", [128, kt_n], i32,
                            kind="ExternalInput").ap()
    out_d = nc.dram_tensor("out", [sq, d], bf16, kind="ExternalOutput").ap()

    with tile.TileContext(nc) as tc, ExitStack() as ctx:
        const = ctx.enter_context(tc.tile_pool(name="const", bufs=1))
        wpool = ctx.enter_context(tc.tile_pool(name="wpool", bufs=1))
        xvp = ctx.enter_context(tc.tile_pool(name="xvp", bufs=3))
        xkp = ctx.enter_context(tc.tile_pool(name="xkp", bufs=3))
        xqp = ctx.enter_context(tc.tile_pool(name="xqp", bufs=3))
        ptp = ctx.enter_context(tc.tile_pool(name="ptp", bufs=6))
        outp = ctx.enter_context(tc.tile_pool(name="outp", bufs=2))
        bcp = ctx.enter_context(tc.tile_pool(name="bcp", bufs=1))
        dnp = ctx.enter_context(tc.tile_pool(name="dnp", bufs=1))
        acc = ctx.enter_context(tc.tile_pool(name="acc", bufs=1, space="PSUM"))
        sblk = ctx.enter_context(tc.tile_pool(name="sblk", bufs=2,
                                              space="PSUM"))
        ctxq = ctx.enter_context(tc.tile_pool(name="ctxq", bufs=3,
                                              space="PSUM"))

        # ---------------- constants / persistent tensors
        ones_f = const.tile([1, 64], f32, tag="ones_f")
        nc.vector.memset(ones_f[:], 1.0)
        ones_sb = const.tile([1, 64], bf16, tag="ones")
        nc.vector.tensor_copy(ones_sb[:], ones_f[:])
        # parity masks: select one 64-partition half, zero the other
        pmask = [const.tile([128, 1], f32, tag=f"pm{i}", name=f"pm{i}")
                 for i in range(2)]
        for i in range(2):
            nc.vector.memset(pmask[i][:], 1.0)
            nc.vector.memset(pmask[i][64 * (1 - i):64 * (2 - i), :], 0.0)
        # constant lower-triangle [k, q'] = (q' >= k) for the diagonal block
        tri = const.tile([128, 128], bf16, tag="tri")
        nc.vector.memset(tri[:], 1.0)
        nc.gpsimd.affine_select(out=tri[:], in_=tri[:], compare_op=is_ge,
                                fill=0.0, base=0, channel_multiplier=-1,
                                pattern=[[1, 128]])
        # per-head 128-partition q/k slots: head j occupies partitions
        # (j%2)*64..+64 of slot j, the other half zeroed via the parity
        # masks at eviction, so score matmuls contract over a full K=128
        qT_sb = const.tile([128, hpc, sq], bf16, tag="qT")
        kT_sb = const.tile([128, hpc, sk], bf16, tag="kT")
        v_sb = const.tile([128, kt_n, hpc, vw], bf16, tag="v")
        cxa = [const.tile([128, sq], bf16, tag=f"cx{m}", name=f"cx{m}")
               for m in range(mc_n)]

        # per-token-chunk x loads: four contiguous 256KB DMAs (ktile
        # granular, so the first projection matmuls start after 1/4 of
        # the chunk has landed)
        def load_x(pool, x_dram, tc):
            t = pool.tile([128, 4, kc_n, 128], bf16, tag="x", name="xc")
            for tl in range(4):
                nc.sync.dma_start(t[:, tl], x_dram[:, tc, tl])
            return t

        # DMAs in first-use order: V-proj(0) only needs wv + mask + xv0,
        # so those go first; wo (needed last) goes behind the chunk-0/1
        # operands
        wv_sb = wpool.tile([128, kc_n, dkc], bf16, tag="wv")
        wv_h = max(1, kc_n // 2)
        nc.sync.dma_start(wv_sb[:, 0:wv_h], wv_d[:, 0:wv_h])
        mask_i = const.tile([128, kt_n], i32, tag="mask_i")
        nc.sync.dma_start(mask_i[:], mask_d)
        xv0 = xvp.tile([128, 4, kc_n, 128], bf16, tag="x", name="xc")
        nc.sync.dma_start(xv0[:, 0], xvT[:, 0, 0])
        nc.sync.dma_start(wv_sb[:, wv_h:kc_n], wv_d[:, wv_h:kc_n])
        for tl in range(1, 4):
            nc.sync.dma_start(xv0[:, tl], xvT[:, 0, tl])
        wk_sb = wpool.tile([128, kc_n, dkc], bf16, tag="wk")
        nc.sync.dma_start(wk_sb[:], wk_d)
        xk0 = load_x(xkp, xkT, 0)
        wq_sb = wpool.tile([128, kc_n, dkc], bf16, tag="wq")
        nc.sync.dma_start(wq_sb[:], wq_d)
        xq0 = load_x(xqp, xqT, 0)
        wo_sb = wpool.tile([128, mc_n, fc_n, 512], bf16, tag="wo")
        nc.sync.dma_start(wo_sb[:], wo_d)
        mask01 = const.tile([128, kt_n], f32, tag="mask01")
        nc.vector.tensor_copy(mask01[:], mask_i[:])
        mask01p = const.tile([128, kt_n], f32, tag="mask01p")
        nc.vector.tensor_scalar_add(mask01p[:], mask01[:], ONES_EPS)

        # ---------------- V projection, one ktile (mask folded in)
        def vproj_kt(xv, tc, tl):
            t = tc * 4 + tl
            pvp = acc if tl % 2 == 0 else sblk
            pv = pvp.tile([128, dkc], f32,
                          tag="acc" if tl % 2 == 0 else "s", name="pv")
            for c in range(kc_n):
                nc.tensor.matmul(pv[:], xv[:, tl, c, :], wv_sb[:, c, :],
                                 start=(c == 0), stop=(c == kc_n - 1))
            nc.vector.tensor_scalar(
                out=v_sb[:, t, :, 0:DK],
                in0=pv[:].rearrange("p (h k) -> p h k", h=hpc),
                scalar1=mask01[:, t:t + 1], scalar2=None, op0=mult)
            nc.vector.tensor_copy(
                v_sb[:, t, :, DK:vw],
                mask01p[:, t:t + 1].unsqueeze(1).broadcast_to([128, hpc, 1]))

        # ---------------- K/Q projection, one 128-feature block
        def proj_m(xs, w_sb, dst, tc, m):
            pkp = acc if m % 2 == 0 else sblk
            pk = pkp.tile([128, 512], f32,
                          tag="acc" if m % 2 == 0 else "s", name="pk")
            for c in range(kc_n):
                nc.tensor.matmul(
                    pk[:], w_sb[:, c, m * 128:(m + 1) * 128],
                    xs[:, :, c, :],
                    start=(c == 0), stop=(c == kc_n - 1))
            qs = slice(tc * 512, tc * 512 + 512)
            nc.scalar.mul(dst[:, 2 * m, qs], pk[:], pmask[0][:])
            nc.vector.tensor_scalar(
                out=dst[:, 2 * m + 1, qs], in0=pk[:],
                scalar1=pmask[1][:], scalar2=None, op0=mult)

        # ---------------- attention, q-chunk major, exact causal trim.
        # Per q-chunk the (head, unit) work items are flattened into one
        # list and the AV matmuls of unit i are emitted after the score
        # matmuls of unit i+2 (and the head-pair normalization inside the
        # next pair), so the exp chain hides under later score matmuls.
        def attention_qc(qc, fillers=(), carry=(), flush=True):
            q0 = qc * QCH
            ktn0 = q0 // 128                  # full (pre-diagonal) ktiles
            deferred = list(carry)
            fillers = list(fillers)
            n_units = hpc * (ktn0 // 2 + 4)
            nf = len(fillers)
            state = [0, 0]                    # units done, fillers emitted
            denom = n_units

            def tick():
                # spread the independent filler work (next chunk's
                # projections, previous chunk's O-proj) evenly between
                # attention units so exp/normalize chains never idle the PE
                state[0] += 1
                want = min(nf, nf * state[0] // denom)
                while state[1] < want:
                    fillers[state[1]]()
                    state[1] += 1

            def mk_av_full(cx_ps, pB, j, blk):
                def go():
                    for t2 in range(2):
                        kt = blk * 2 + t2
                        nc.tensor.matmul(cx_ps[:], v_sb[:, kt, j, :],
                                         pB[:, t2, :],
                                         start=(kt == 0), stop=False)
                return go

            def mk_av_diag(cx_ps, pB, j, dd):
                kt = ktn0 + dd
                qlo = dd * 128
                n = QCH - qlo
                def go():
                    nc.tensor.matmul(cx_ps[:, qlo:QCH], v_sb[:, kt, j, :],
                                     pB[:, 0, 0:n],
                                     start=(kt == 0), stop=(dd == 3))
                return go

            def mk_norm_pair(cx_even, cx_odd, j_odd):
                def go():
                    ms = j_odd // 2
                    dn_e = dnp.tile([1, QCH], bf16, tag="dne", name="dne")
                    dn_o = dnp.tile([1, QCH], bf16, tag="dno", name="dno")
                    nc.vector.tensor_copy(dn_e[:], cx_even[DK:DK + 1, :])
                    nc.vector.tensor_copy(dn_o[:], cx_odd[DK:DK + 1, :])
                    bc_ps = acc.tile([128, QCH], f32, tag="acc", name="bc_ps")
                    nc.tensor.matmul(bc_ps[0:64, :], ones_sb[:], dn_e[:],
                                     start=True, stop=True)
                    nc.tensor.matmul(bc_ps[64:128, :], ones_sb[:], dn_o[:],
                                     start=True, stop=True)
                    bc = bcp.tile([128, QCH], f32, tag="bc", name="bc")
                    nc.vector.reciprocal_approx_fast(bc[:], bc_ps[:])
                    nc.vector.tensor_tensor(
                        out=cxa[ms][0:64, q0:q0 + QCH],
                        in0=cx_even[0:DK, :], in1=bc[0:64, :], op=mult)
                    nc.vector.tensor_tensor(
                        out=cxa[ms][64:128, q0:q0 + QCH],
                        in0=cx_odd[0:DK, :], in1=bc[64:128, :], op=mult)
                return go

            def emit_full(cx_ps, j, blk):
                sB = sblk.tile([128, 2, 512], f32, tag="s", name="sB")
                for t2 in range(2):
                    kt = blk * 2 + t2
                    nc.tensor.matmul(
                        sB[:, t2, :],
                        kT_sb[:, j, kt * 128:(kt + 1) * 128],
                        qT_sb[:, j, q0:q0 + QCH],
                        start=True, stop=True)
                pB = ptp.tile([128, 2, 512], bf16, tag="p", name="pB")
                nc.scalar.activation(pB[:], sB[:], Exp, scale=0.125)
                deferred.append(mk_av_full(cx_ps, pB, j, blk))

            def emit_diag(cx_ps, j, dd):
                kt = ktn0 + dd
                qlo = dd * 128
                n = QCH - qlo
                sB = sblk.tile([128, 2, 512], f32, tag="s", name="sB")
                nc.tensor.matmul(
                    sB[:, 0, 0:n],
                    kT_sb[:, j, kt * 128:(kt + 1) * 128],
                    qT_sb[:, j, q0 + qlo:q0 + QCH],
                    start=True, stop=True)
                pB = ptp.tile([128, 2, 512], bf16, tag="p", name="pB")
                nc.scalar.activation(pB[:, 0, 0:n], sB[:, 0, 0:n],
                                     Exp, scale=0.125)
                # causal mask of the partial 128x128 diagonal block
                nc.vector.tensor_tensor(out=pB[:, 0, 0:128],
                                        in0=pB[:, 0, 0:128],
                                        in1=tri[:], op=mult)
                deferred.append(mk_av_diag(cx_ps, pB, j, dd))

            cx_prev = None
            for j in range(hpc):
                cx_ps = ctxq.tile([vw, QCH], f32, tag="ctx", name="cx_ps")
                # interleave the small diagonal units among the full
                # blocks so PE work per unit stays uniform and the
                # deferred-AV window doesn't collapse at the head tail.
                # Accumulation order is commutative; blk 0 (which writes
                # every column with start=True) stays first and diag 3
                # (stop=True) stays last.
                nblk = ktn0 // 2
                order = ([('f', b) for b in range(nblk)]
                         + [('d', k) for k in range(4)])
                for kind, idx in order:
                    if kind == 'f':
                        emit_full(cx_ps, j, idx)
                    else:
                        emit_diag(cx_ps, j, idx)
                    while len(deferred) > 3:
                        deferred.pop(0)()
                    tick()
                if j % 2 == 0:
                    cx_prev = cx_ps
                else:
                    deferred.append(mk_norm_pair(cx_prev, cx_ps, j))
            while state[1] < nf:
                fillers[state[1]]()
                state[1] += 1
            if flush:
                for fn in deferred:
                    fn()
                return []
            # carry the trailing AVs + final pair-norm into the next
            # chunk's deferred stream so their dependency chains hide
            # under its score matmuls instead of stalling the boundary
            return deferred

        def oproj_qt(qc, qt, last=False):
            qg = qc * QCH + qt * 128
            po = sblk.tile([128, fc_n, 512], f32, tag="s", name="po")
            for fc in range(fc_n):
                for m in range(mc_n):
                    nc.tensor.matmul(
                        po[:, fc, :], cxa[m][:, qg:qg + 128],
                        wo_sb[:, m, fc, :],
                        start=(m == 0), stop=(m == mc_n - 1))
            o_sb = outp.tile([128, fc_n, 512], bf16, tag="o", name="o_sb")
            # in the tail both scalar and vector are idle: alternate so
            # consecutive evictions don't serialize on one engine
            if last and qt % 2 == 0:
                nc.scalar.copy(o_sb[:], po[:])
            else:
                nc.vector.tensor_copy(o_sb[:], po[:])
            nc.sync.dma_start(out_d[qg:qg + 128, :],
                              o_sb[:].rearrange("p f n -> p (f n)"))

        # ---------------- token-chunk pipeline
        # chunk-0 projections up front; for each chunk the projections of
        # tc+1 and the O-projection of tc-1 are emitted as fillers woven
        # between attention(tc) units
        xv_t = [None] * qc_n
        xk_t = [None] * qc_n
        xq_t = [None] * qc_n
        xv_t[0], xk_t[0], xq_t[0] = xv0, xk0, xq0
        if qc_n > 1:
            xv_t[1] = load_x(xvp, xvT, 1)
            xk_t[1] = load_x(xkp, xkT, 1)
            xq_t[1] = load_x(xqp, xqT, 1)
        for tl in range(4):
            vproj_kt(xv_t[0], 0, tl)
        for m in range(mc_n):
            proj_m(xk_t[0], wk_sb, kT_sb, 0, m)
        for m in range(mc_n):
            proj_m(xq_t[0], wq_sb, qT_sb, 0, m)
        carry = []
        for tc in range(qc_n):
            if tc + 2 < qc_n:
                xv_t[tc + 2] = load_x(xvp, xvT, tc + 2)
                xk_t[tc + 2] = load_x(xkp, xkT, tc + 2)
                xq_t[tc + 2] = load_x(xqp, xqT, tc + 2)
            fillers = []
            if tc + 1 < qc_n:
                nt = tc + 1
                fillers += [
                    (lambda tl=tl, t=nt: vproj_kt(xv_t[t], t, tl))
                    for tl in range(4)]
            if tc >= 1:
                fillers += [
                    (lambda qt=qt, q=tc - 1: oproj_qt(q, qt))
                    for qt in range(QCH // 128)]
            if tc + 1 < qc_n:
                nt = tc + 1
                fillers += [
                    (lambda m=m, t=nt: proj_m(xk_t[t], wk_sb, kT_sb, t, m))
                    for m in range(mc_n)]
                fillers += [
                    (lambda m=m, t=nt: proj_m(xq_t[t], wq_sb, qT_sb, t, m))
                    for m in range(mc_n)]
            carry = attention_qc(tc, fillers)
        for qt in range(QCH // 128):
            oproj_qt(qc_n - 1, qt, last=True)
    nc.compile()
    return nc


def _get_program(cfg):
    if cfg not in _PROG_CACHE:
        _PROG_CACHE[cfg] = _build(cfg)
    return _PROG_CACHE[cfg]


def _dev_x(xT, bf):
    """[d, tok] -> [128, tok//512, 4, d//128, 128] (device DMA layout,
    ktile-block granular so loads land 256KB at a time)."""
    d, tok = xT.shape
    a = (xT.reshape(d // 128, 128, tok // 512, 4, 128)
         .transpose(1, 2, 3, 0, 4))
    return np.ascontiguousarray(a.astype(bf))


def _dev_w(wT, bf):
    """[d, dkc] -> [128, d//128, dkc]."""
    d, dkc = wT.shape
    a = wT.reshape(d // 128, 128, dkc).transpose(1, 0, 2)
    return np.ascontiguousarray(a.astype(bf))


def _dev_wo(woT, bf):
    """[dkc, d] -> [128, dkc//128, d//512, 512]."""
    dkc, d = woT.shape
    a = woT.reshape(dkc // 128, 128, d // 512, 512).transpose(1, 0, 2, 3)
    return np.ascontiguousarray(a.astype(bf))


def _dev_mask(mask):
    """[sk] -> [128, sk//128] with element t*128+p at [p, t]."""
    return np.ascontiguousarray(
        mask.reshape(-1, 128).T.astype(np.int32))


def _shard_inputs(query, key, value, mask, Wq, Wk, Wv, Wo):
    """Build the 8 per-core input maps (bf16 device payloads)."""
    import ml_dtypes
    bf = ml_dtypes.bfloat16
    in_maps = []
    xt = {}
    for b in range(B):
        xt[b] = (_dev_x(query[b].T, bf), _dev_x(key[b].T, bf),
                 _dev_x(value[b].T, bf), _dev_mask(mask[b]))
    for c in range(N_CORES):
        b, hg = divmod(c, CORES_PER_BATCH)
        rows = slice(hg * DKC, (hg + 1) * DKC)
        xq, xk, xv, mb = xt[b]
        in_maps.append({
            "xqT": xq, "xkT": xk, "xvT": xv, "maskb": mb,
            "wq": _dev_w(Wq[rows, :].T, bf),
            "wk": _dev_w(Wk[rows, :].T, bf),
            "wv": _dev_w(Wv[rows, :].T, bf),
            "wo": _dev_wo(Wo[:, rows].T, bf),
        })
    return in_maps


def kernel(query, key, value, mask, Wq, Wk, Wv, Wo):
    from concourse.bass_utils import run_bass_kernel_spmd

    nc = _get_program((SQ, SK, D, DKC))
    in_maps = _shard_inputs(np.asarray(query), np.asarray(key),
                            np.asarray(value), np.asarray(mask),
                            np.asarray(Wq), np.asarray(Wk),
                            np.asarray(Wv), np.asarray(Wo))
    res = run_bass_kernel_spmd(nc, in_maps, list(range(N_CORES)))
    out = np.zeros((B, SQ, D), dtype=np.float32)
    for c in range(N_CORES):
        out[c // CORES_PER_BATCH] += res.results[c]["out"].astype(np.float32)
    return out


# revision 59
# speedup vs baseline: 1.0039x; 1.0039x over previous
"""Multi-head attention (B=2, SQ=SK=2048, D=1024, H=16, DK=64) on 8 TRN2 cores.

Sharding: core c handles batch b = c//4 and head-group hg = c%4 (4 heads,
256 feature columns of each projection).  Each core computes its heads'
Q/K/V projections, causal+padding-masked softmax attention, and a partial
output projection; the host sums the 4 partials per batch.

v4: token-chunk software pipeline.  The per-512-token-chunk order is
  V-proj(tc) -> K-proj(tc) -> Q-proj(tc) -> O-proj(tc-1) -> attention(tc)
which is exact because causal attention for q-chunk tc only consumes
K/V tokens <= (tc+1)*512.  This overlaps the exp stream (scalar engine,
the co-bottleneck) with projection matmuls instead of serializing all
projections first.

All matmul operands are bf16 (fp32 PSUM accumulation); the host
pre-converts inputs and upconverts/sums the bf16 output partials.
Exact causal trimming: the 4 diagonal ktiles of every q-chunk only
compute the valid q tail (N = 512-128*d); the partial 128x128 diagonal
block is masked by a DVE multiply with a constant lower-triangle tile.

Device layouts (per core):
  qT/kT  [dk, tok]    dk on partitions, produced directly by the projection
  v      [tok, dk]    natural, padding mask folded into the rows plus a
                      "masked ones" column per head (the ones column makes
                      the ctxT matmul emit the softmax denominator for free)
  sT     [ktok, qtok] transposed scores (PSUM)
  pT     exp(sT/8)    SBUF bf16
  ctxT   [dk+1, qtok] accumulated over ktok tiles (last row = denominator)
  out    [qtok, D]    ctxT is the stationary operand, both sides natural

Softmax runs without max subtraction (scores are O(6) for randn inputs, so
exp cannot overflow).  Padding is exact: masked keys contribute exactly
zero to numerator and denominator, and all-masked rows produce ~0 output
(matching the reference's nan_to_num) via a tiny epsilon in the ones
column.
"""

import numpy as np

B, SQ, SK, D, H, DK = 2, 2048, 2048, 1024, 16, 64
N_CORES = 8
CORES_PER_BATCH = 4
DKC = D // CORES_PER_BATCH          # 256 projection columns per core
QCH = 512                           # q-chunk (moving free dim)
ONES_EPS = 1e-20

_PROG_CACHE = {}


def _build(cfg):
    """Build the per-core Bass program. cfg = (sq, sk, d, dkc)."""
    import concourse.bass as bass  # noqa: F401
    import concourse.mybir as mybir
    import concourse.tile as tile
    from concourse import bacc
    from contextlib import ExitStack

    f32 = mybir.dt.float32
    bf16 = mybir.dt.bfloat16
    i32 = mybir.dt.int32
    Exp = mybir.ActivationFunctionType.Exp
    mult = mybir.AluOpType.mult
    is_ge = mybir.AluOpType.is_ge

    sq, sk, d, dkc = cfg
    kc_n = d // 128                  # contraction chunks for projections
    mc_n = dkc // 128                # 128-wide dk chunks (q/k layout)
    kt_n = sk // 128                 # key tiles
    qc_n = sq // QCH                 # q chunks
    hpc = dkc // DK                  # heads per core
    vw = DK + 1                      # v row width per head incl. ones col
    fc_n = d // 512                  # output feature chunks

    nc = bacc.Bacc("TRN2", target_bir_lowering=False, debug=False,
                   enable_asserts=False, num_devices=N_CORES)

    # all inputs are pre-arranged on the host into the exact device layout
    # so every DMA is a single fully-contiguous descriptor (>=4KB runs per
    # partition; the former rearrange patterns ran the DMA engines at half
    # rate on 512B-1KB runs)
    xqT = nc.dram_tensor("xqT", [128, sq // 512, 4, kc_n, 128], bf16,
                         kind="ExternalInput").ap()
    xkT = nc.dram_tensor("xkT", [128, sk // 512, 4, kc_n, 128], bf16,
                         kind="ExternalInput").ap()
    xvT = nc.dram_tensor("xvT", [128, sk // 512, 4, kc_n, 128], bf16,
                         kind="ExternalInput").ap()
    wq_d = nc.dram_tensor("wq", [128, kc_n, dkc], bf16,
                          kind="ExternalInput").ap()
    wk_d = nc.dram_tensor("wk", [128, kc_n, dkc], bf16,
                          kind="ExternalInput").ap()
    wv_d = nc.dram_tensor("wv", [128, kc_n, dkc], bf16,
                          kind="ExternalInput").ap()
    wo_d = nc.dram_tensor("wo", [128, mc_n, fc_n, 512], bf16,
                          kind="ExternalInput").ap()
    mask_d = nc.dram_tensor("maskb", [128, kt_n], i32,
                            kind="ExternalInput").ap()
    out_d = nc.dram_tensor("out", [sq, d], bf16, kind="ExternalOutput").ap()

    with tile.TileContext(nc) as tc, ExitStack() as ctx:
        const = ctx.enter_context(tc.tile_pool(name="const", bufs=1))
        wpool = ctx.enter_context(tc.tile_pool(name="wpool", bufs=1))
        xvp = ctx.enter_context(tc.tile_pool(name="xvp", bufs=3))
        xkp = ctx.enter_context(tc.tile_pool(name="xkp", bufs=3))
        xqp = ctx.enter_context(tc.tile_pool(name="xqp", bufs=3))
        ptp = ctx.enter_context(tc.tile_pool(name="ptp", bufs=6))
        outp = ctx.enter_context(tc.tile_pool(name="outp", bufs=2))
        bcp = ctx.enter_context(tc.tile_pool(name="bcp", bufs=1))
        dnp = ctx.enter_context(tc.tile_pool(name="dnp", bufs=1))
        acc = ctx.enter_context(tc.tile_pool(name="acc", bufs=1, space="PSUM"))
        sblk = ctx.enter_context(tc.tile_pool(name="sblk", bufs=2,
                                              space="PSUM"))
        ctxq = ctx.enter_context(tc.tile_pool(name="ctxq", bufs=3,
                                              space="PSUM"))

        # ---------------- constants / persistent tensors
        ones_f = const.tile([1, 64], f32, tag="ones_f")
        nc.vector.memset(ones_f[:], 1.0)
        ones_sb = const.tile([1, 64], bf16, tag="ones")
        nc.vector.tensor_copy(ones_sb[:], ones_f[:])
        # parity masks: select one 64-partition half, zero the other
        pmask = [const.tile([128, 1], f32, tag=f"pm{i}", name=f"pm{i}")
                 for i in range(2)]
        for i in range(2):
            nc.vector.memset(pmask[i][:], 1.0)
            nc.vector.memset(pmask[i][64 * (1 - i):64 * (2 - i), :], 0.0)
        # constant lower-triangle [k, q'] = (q' >= k) for the diagonal block
        tri = const.tile([128, 128], bf16, tag="tri")
        nc.vector.memset(tri[:], 1.0)
        nc.gpsimd.affine_select(out=tri[:], in_=tri[:], compare_op=is_ge,
                                fill=0.0, base=0, channel_multiplier=-1,
                                pattern=[[1, 128]])
        # per-head 128-partition q/k slots: head j occupies partitions
        # (j%2)*64..+64 of slot j, the other half zeroed via the parity
        # masks at eviction, so score matmuls contract over a full K=128
        qT_sb = const.tile([128, hpc, sq], bf16, tag="qT")
        kT_sb = const.tile([128, hpc, sk], bf16, tag="kT")
        v_sb = const.tile([128, kt_n, hpc, vw], bf16, tag="v")
        cxa = [const.tile([128, sq], bf16, tag=f"cx{m}", name=f"cx{m}")
               for m in range(mc_n)]

        # per-token-chunk x loads: four contiguous 256KB DMAs (ktile
        # granular, so the first projection matmuls start after 1/4 of
        # the chunk has landed)
        def load_x(pool, x_dram, tc):
            t = pool.tile([128, 4, kc_n, 128], bf16, tag="x", name="xc")
            for tl in range(4):
                nc.sync.dma_start(t[:, tl], x_dram[:, tc, tl])
            return t

        # DMAs in first-use order: V-proj(0) only needs wv + mask + xv0,
        # so those go first; wo (needed last) goes behind the chunk-0/1
        # operands
        wv_sb = wpool.tile([128, kc_n, dkc], bf16, tag="wv")
        nc.sync.dma_start(wv_sb[:], wv_d)
        mask_i = const.tile([128, kt_n], i32, tag="mask_i")
        nc.sync.dma_start(mask_i[:], mask_d)
        xv0 = load_x(xvp, xvT, 0)
        wk_sb = wpool.tile([128, kc_n, dkc], bf16, tag="wk")
        nc.sync.dma_start(wk_sb[:], wk_d)
        xk0 = load_x(xkp, xkT, 0)
        wq_sb = wpool.tile([128, kc_n, dkc], bf16, tag="wq")
        nc.sync.dma_start(wq_sb[:], wq_d)
        xq0 = load_x(xqp, xqT, 0)
        wo_sb = wpool.tile([128, mc_n, fc_n, 512], bf16, tag="wo")
        nc.sync.dma_start(wo_sb[:], wo_d)
        mask01 = const.tile([128, kt_n], f32, tag="mask01")
        nc.vector.tensor_copy(mask01[:], mask_i[:])
        mask01p = const.tile([128, kt_n], f32, tag="mask01p")
        nc.vector.tensor_scalar_add(mask01p[:], mask01[:], ONES_EPS)

        # ---------------- V projection, one ktile (mask folded in)
        def vproj_kt(xv, tc, tl):
            t = tc * 4 + tl
            pvp = acc if tl % 2 == 0 else sblk
            pv = pvp.tile([128, dkc], f32,
                          tag="acc" if tl % 2 == 0 else "s", name="pv")
            for c in range(kc_n):
                nc.tensor.matmul(pv[:], xv[:, tl, c, :], wv_sb[:, c, :],
                                 start=(c == 0), stop=(c == kc_n - 1))
            nc.vector.tensor_scalar(
                out=v_sb[:, t, :, 0:DK],
                in0=pv[:].rearrange("p (h k) -> p h k", h=hpc),
                scalar1=mask01[:, t:t + 1], scalar2=None, op0=mult)
            nc.vector.tensor_copy(
                v_sb[:, t, :, DK:vw],
                mask01p[:, t:t + 1].unsqueeze(1).broadcast_to([128, hpc, 1]))

        # ---------------- K/Q projection, one 128-feature block
        def proj_m(xs, w_sb, dst, tc, m):
            pkp = acc if m % 2 == 0 else sblk
            pk = pkp.tile([128, 512], f32,
                          tag="acc" if m % 2 == 0 else "s", name="pk")
            for c in range(kc_n):
                nc.tensor.matmul(
                    pk[:], w_sb[:, c, m * 128:(m + 1) * 128],
                    xs[:, :, c, :],
                    start=(c == 0), stop=(c == kc_n - 1))
            qs = slice(tc * 512, tc * 512 + 512)
            nc.scalar.mul(dst[:, 2 * m, qs], pk[:], pmask[0][:])
            nc.vector.tensor_scalar(
                out=dst[:, 2 * m + 1, qs], in0=pk[:],
                scalar1=pmask[1][:], scalar2=None, op0=mult)

        # ---------------- attention, q-chunk major, exact causal trim.
        # Per q-chunk the (head, unit) work items are flattened into one
        # list and the AV matmuls of unit i are emitted after the score
        # matmuls of unit i+2 (and the head-pair normalization inside the
        # next pair), so the exp chain hides under later score matmuls.
        def attention_qc(qc, fillers=(), carry=(), flush=True):
            q0 = qc * QCH
            ktn0 = q0 // 128                  # full (pre-diagonal) ktiles
            deferred = list(carry)
            fillers = list(fillers)
            n_units = hpc * (ktn0 // 2 + 4)
            nf = len(fillers)
            state = [0, 0]                    # units done, fillers emitted
            denom = n_units

            def tick():
                # spread the independent filler work (next chunk's
                # projections, previous chunk's O-proj) evenly between
                # attention units so exp/normalize chains never idle the PE
                state[0] += 1
                want = min(nf, nf * state[0] // denom)
                while state[1] < want:
                    fillers[state[1]]()
                    state[1] += 1

            def mk_av_full(cx_ps, pB, j, blk):
                def go():
                    for t2 in range(2):
                        kt = blk * 2 + t2
                        nc.tensor.matmul(cx_ps[:], v_sb[:, kt, j, :],
                                         pB[:, t2, :],
                                         start=(kt == 0), stop=False)
                return go

            def mk_av_diag(cx_ps, pB, j, dd):
                kt = ktn0 + dd
                qlo = dd * 128
                n = QCH - qlo
                def go():
                    nc.tensor.matmul(cx_ps[:, qlo:QCH], v_sb[:, kt, j, :],
                                     pB[:, 0, 0:n],
                                     start=(kt == 0), stop=(dd == 3))
                return go

            def mk_norm_pair(cx_even, cx_odd, j_odd):
                def go():
                    ms = j_odd // 2
                    dn_e = dnp.tile([1, QCH], bf16, tag="dne", name="dne")
                    dn_o = dnp.tile([1, QCH], bf16, tag="dno", name="dno")
                    nc.vector.tensor_copy(dn_e[:], cx_even[DK:DK + 1, :])
                    nc.vector.tensor_copy(dn_o[:], cx_odd[DK:DK + 1, :])
                    bc_ps = acc.tile([128, QCH], f32, tag="acc", name="bc_ps")
                    nc.tensor.matmul(bc_ps[0:64, :], ones_sb[:], dn_e[:],
                                     start=True, stop=True)
                    nc.tensor.matmul(bc_ps[64:128, :], ones_sb[:], dn_o[:],
                                     start=True, stop=True)
                    bc = bcp.tile([128, QCH], f32, tag="bc", name="bc")
                    nc.vector.reciprocal_approx_fast(bc[:], bc_ps[:])
                    nc.vector.tensor_tensor(
                        out=cxa[ms][0:64, q0:q0 + QCH],
                        in0=cx_even[0:DK, :], in1=bc[0:64, :], op=mult)
                    nc.vector.tensor_tensor(
                        out=cxa[ms][64:128, q0:q0 + QCH],
                        in0=cx_odd[0:DK, :], in1=bc[64:128, :], op=mult)
                return go

            def emit_full(cx_ps, j, blk):
                sB = sblk.tile([128, 2, 512], f32, tag="s", name="sB")
                for t2 in range(2):
                    kt = blk * 2 + t2
                    nc.tensor.matmul(
                        sB[:, t2, :],
                        kT_sb[:, j, kt * 128:(kt + 1) * 128],
                        qT_sb[:, j, q0:q0 + QCH],
                        start=True, stop=True)
                pB = ptp.tile([128, 2, 512], bf16, tag="p", name="pB")
                nc.scalar.activation(pB[:], sB[:], Exp, scale=0.125)
                deferred.append(mk_av_full(cx_ps, pB, j, blk))

            def emit_diag(cx_ps, j, dd):
                kt = ktn0 + dd
                qlo = dd * 128
                n = QCH - qlo
                sB = sblk.tile([128, 2, 512], f32, tag="s", name="sB")
                nc.tensor.matmul(
                    sB[:, 0, 0:n],
                    kT_sb[:, j, kt * 128:(kt + 1) * 128],
                    qT_sb[:, j, q0 + qlo:q0 + QCH],
                    start=True, stop=True)
                pB = ptp.tile([128, 2, 512], bf16, tag="p", name="pB")
                nc.scalar.activation(pB[:, 0, 0:n], sB[:, 0, 0:n],
                                     Exp, scale=0.125)
                # causal mask of the partial 128x128 diagonal block
                nc.vector.tensor_tensor(out=pB[:, 0, 0:128],
                                        in0=pB[:, 0, 0:128],
                                        in1=tri[:], op=mult)
                deferred.append(mk_av_diag(cx_ps, pB, j, dd))

            cx_prev = None
            for j in range(hpc):
                cx_ps = ctxq.tile([vw, QCH], f32, tag="ctx", name="cx_ps")
                # interleave the small diagonal units among the full
                # blocks so PE work per unit stays uniform and the
                # deferred-AV window doesn't collapse at the head tail.
                # Accumulation order is commutative; blk 0 (which writes
                # every column with start=True) stays first and diag 3
                # (stop=True) stays last.
                nblk = ktn0 // 2
                order = ([('f', b) for b in range(nblk)]
                         + [('d', k) for k in range(4)])
                for kind, idx in order:
                    if kind == 'f':
                        emit_full(cx_ps, j, idx)
                    else:
                        emit_diag(cx_ps, j, idx)
                    while len(deferred) > 3:
                        deferred.pop(0)()
                    tick()
                if j % 2 == 0:
                    cx_prev = cx_ps
                else:
                    deferred.append(mk_norm_pair(cx_prev, cx_ps, j))
            while state[1] < nf:
                fillers[state[1]]()
                state[1] += 1
            if flush:
                # dependency-free warm-up matmuls between the trailing
                # AVs, the normalization, and the closing O-projection:
                # they fill the PE-idle windows of the final normalize
                # chain so the PE stays at full pstate for the O-proj
                # (measured ~600ns vs ~380ns per matmul when it ramps
                # from cold)
                def warm(n):
                    for _ in range(n):
                        wt = sblk.tile([128, 2, 512], f32, tag="s",
                                       name="warm")
                        nc.tensor.matmul(wt[0:64, 0, :], ones_sb[:],
                                         qT_sb[0:1, 0, 0:512],
                                         start=True, stop=True)
                for fn in deferred[:-1]:
                    fn()
                warm(2)
                deferred[-1]()
                warm(2)
                return []
            # carry the trailing AVs + final pair-norm into the next
            # chunk's deferred stream so their dependency chains hide
            # under its score matmuls instead of stalling the boundary
            return deferred

        def oproj_qt(qc, qt, last=False):
            qg = qc * QCH + qt * 128
            po = sblk.tile([128, fc_n, 512], f32, tag="s", name="po")
            for fc in range(fc_n):
                for m in range(mc_n):
                    nc.tensor.matmul(
                        po[:, fc, :], cxa[m][:, qg:qg + 128],
                        wo_sb[:, m, fc, :],
                        start=(m == 0), stop=(m == mc_n - 1))
            o_sb = outp.tile([128, fc_n, 512], bf16, tag="o", name="o_sb")
            # in the tail both scalar and vector are idle: alternate so
            # consecutive evictions don't serialize on one engine
            if last and qt % 2 == 0:
                nc.scalar.copy(o_sb[:], po[:])
            else:
                nc.vector.tensor_copy(o_sb[:], po[:])
            nc.sync.dma_start(out_d[qg:qg + 128, :],
                              o_sb[:].rearrange("p f n -> p (f n)"))

        # ---------------- token-chunk pipeline
        # chunk-0 projections up front; for each chunk the projections of
        # tc+1 and the O-projection of tc-1 are emitted as fillers woven
        # between attention(tc) units
        xv_t = [None] * qc_n
        xk_t = [None] * qc_n
        xq_t = [None] * qc_n
        xv_t[0], xk_t[0], xq_t[0] = xv0, xk0, xq0
        if qc_n > 1:
            xv_t[1] = load_x(xvp, xvT, 1)
            xk_t[1] = load_x(xkp, xkT, 1)
            xq_t[1] = load_x(xqp, xqT, 1)
        for tl in range(4):
            vproj_kt(xv_t[0], 0, tl)
        for m in range(mc_n):
            proj_m(xk_t[0], wk_sb, kT_sb, 0, m)
        for m in range(mc_n):
            proj_m(xq_t[0], wq_sb, qT_sb, 0, m)
        carry = []
        for tc in range(qc_n):
            if tc + 2 < qc_n:
                xv_t[tc + 2] = load_x(xvp, xvT, tc + 2)
                xk_t[tc + 2] = load_x(xkp, xkT, tc + 2)
                xq_t[tc + 2] = load_x(xqp, xqT, tc + 2)
            fillers = []
            if tc + 1 < qc_n:
                nt = tc + 1
                fillers += [
                    (lambda tl=tl, t=nt: vproj_kt(xv_t[t], t, tl))
                    for tl in range(4)]
            if tc >= 1:
                fillers += [
                    (lambda qt=qt, q=tc - 1: oproj_qt(q, qt))
                    for qt in range(QCH // 128)]
            if tc + 1 < qc_n:
                nt = tc + 1
                fillers += [
                    (lambda m=m, t=nt: proj_m(xk_t[t], wk_sb, kT_sb, t, m))
                    for m in range(mc_n)]
                fillers += [
                    (lambda m=m, t=nt: proj_m(xq_t[t], wq_sb, qT_sb, t, m))
                    for m in range(mc_n)]
            carry = attention_qc(tc, fillers)
        for qt in range(QCH // 128):
            oproj_qt(qc_n - 1, qt, last=True)
    nc.compile()
    return nc


def _get_program(cfg):
    if cfg not in _PROG_CACHE:
        _PROG_CACHE[cfg] = _build(cfg)
    return _PROG_CACHE[cfg]


def _dev_x(xT, bf):
    """[d, tok] -> [128, tok//512, 4, d//128, 128] (device DMA layout,
    ktile-block granular so loads land 256KB at a time)."""
    d, tok = xT.shape
    a = (xT.reshape(d // 128, 128, tok // 512, 4, 128)
         .transpose(1, 2, 3, 0, 4))
    return np.ascontiguousarray(a.astype(bf))


def _dev_w(wT, bf):
    """[d, dkc] -> [128, d//128, dkc]."""
    d, dkc = wT.shape
    a = wT.reshape(d // 128, 128, dkc).transpose(1, 0, 2)
    return np.ascontiguousarray(a.astype(bf))


def _dev_wo(woT, bf):
    """[dkc, d] -> [128, dkc//128, d//512, 512]."""
    dkc, d = woT.shape
    a = woT.reshape(dkc // 128, 128, d // 512, 512).transpose(1, 0, 2, 3)
    return np.ascontiguousarray(a.astype(bf))


def _dev_mask(mask):
    """[sk] -> [128, sk//128] with element t*128+p at [p, t]."""
    return np.ascontiguousarray(
        mask.reshape(-1, 128).T.astype(np.int32))


def _shard_inputs(query, key, value, mask, Wq, Wk, Wv, Wo):
    """Build the 8 per-core input maps (bf16 device payloads)."""
    import ml_dtypes
    bf = ml_dtypes.bfloat16
    in_maps = []
    xt = {}
    for b in range(B):
        xt[b] = (_dev_x(query[b].T, bf), _dev_x(key[b].T, bf),
                 _dev_x(value[b].T, bf), _dev_mask(mask[b]))
    for c in range(N_CORES):
        b, hg = divmod(c, CORES_PER_BATCH)
        rows = slice(hg * DKC, (hg + 1) * DKC)
        xq, xk, xv, mb = xt[b]
        in_maps.append({
            "xqT": xq, "xkT": xk, "xvT": xv, "maskb": mb,
            "wq": _dev_w(Wq[rows, :].T, bf),
            "wk": _dev_w(Wk[rows, :].T, bf),
            "wv": _dev_w(Wv[rows, :].T, bf),
            "wo": _dev_wo(Wo[:, rows].T, bf),
        })
    return in_maps


def kernel(query, key, value, mask, Wq, Wk, Wv, Wo):
    from concourse.bass_utils import run_bass_kernel_spmd

    nc = _get_program((SQ, SK, D, DKC))
    in_maps = _shard_inputs(np.asarray(query), np.asarray(key),
                            np.asarray(value), np.asarray(mask),
                            np.asarray(Wq), np.asarray(Wk),
                            np.asarray(Wv), np.asarray(Wo))
    res = run_bass_kernel_spmd(nc, in_maps, list(range(N_CORES)))
    out = np.zeros((B, SQ, D), dtype=np.float32)
    for c in range(N_CORES):
        out[c // CORES_PER_BATCH] += res.results[c]["out"].astype(np.float32)
    return out
